# revision 6
# baseline (speedup 1.0000x reference)
"""Trainium2 Bass kernel for nn_NodeModel (gnn_message_passing).

Reference computation:
    agg = segment_sum(edge_attr, edge_index[0], N)   # [N, 64]
    h   = relu(concat([x, agg], 1) @ W1 + b1)        # [N, 256]
    out = h @ W2 + b2                                # [N, 64]
(u and batch are unused by the reference.)

Strategy (8 cores, graph-parallel, fp8 edge stream):
  * Host groups each node's edges into PAIRS (2 edges per PE slot).  A
    128-slot chunk therefore carries 256 edges: the stationary matmul
    operand is [128 slots, 128] with the even edge of each pair in
    columns 0:64 and the odd edge in columns 64:128.  One one-hot
    [128 slots, 32 nodes] routes both halves, so psum rows 0:64 and
    64:128 hold two partial aggregates that a single DVE add folds into
    SBUF (the add doubles as the psum->SBUF copy the pipeline needs
    anyway).  Pairing halves both the LDWEIGHTS count and the matmul
    count, and the full 128-column stationary enables FWL.
  * Edges travel as fp8 E3M4 (4 mantissa bits, max 15.5 >> |edge| max
    5.4), quantized on host with per-(node,feature) error feedback so
    each node's SUM of quantized edges is accurate to ~1 ulp of the
    last element instead of sqrt(deg) ulps.  Measured end-to-end
    rel-err 4.8e-3 vs the 2e-2 gate (e4m3+EF fallback: 8.1e-3).
  * Nodes are packed into 32-node windows with a hard 256-pair cap
    (2 chunks of 128 slots per window, uniform), balanced by a greedy
    heap; 416 windows/core.  16 windows = one 512-node supertile.
  * MLP runs in bf16 on the same supertiles (x, W1, h, W2, out all
    bf16; psum fp32): hT = relu(W1.T @ [aggT; xT] + b1), outT = W2.T @
    hT + b2.  x and out ship as bf16, halving their DMA.
"""

import os
import sys
import heapq

for _p in ("/opt/trn_rl_repo", "/root/.axon_site/_ro/trn_rl_repo"):
    if os.path.isdir(_p) and _p not in sys.path:
        sys.path.insert(0, _p)

import numpy as np
import ml_dtypes
from contextlib import ExitStack

import concourse.bass as bass
import concourse.tile as tile
from concourse import bacc, mybir
from concourse.bass_utils import run_bass_kernel_spmd

F32 = mybir.dt.float32
BF16 = mybir.dt.bfloat16
FP8 = mybir.dt.float8e3
FP8_NP = ml_dtypes.float8_e3m4
BF16_NP = ml_dtypes.bfloat16

NCORES = 8
D = 64            # feature dim
H = 256           # hidden dim
O = 64            # output dim
W = 32            # nodes per window
CHUNK = 128       # pair slots per chunk (PE contraction dim)
CHUNKS = 2        # chunks per window
CAP = CHUNK * CHUNKS   # 256 pair slots (512 edges) per window
G = 16            # windows per group (= one supertile)
ST = G * W        # 512-node MLP supertile


class Cfg:
    def __init__(self, n_nodes, total_pairs, extra=0):
        wpc = max(
            (n_nodes + NCORES * W - 1) // (NCORES * W),
            int(np.ceil(total_pairs * 1.03 / (CAP * NCORES))),
        ) + extra
        wpc = ((wpc + G - 1) // G) * G            # supertile-align
        self.WPC = wpc                            # windows per core
        self.NPC = W * wpc                        # node slots per core
        self.NWIN = NCORES * wpc
        self.NGRP = wpc // G                      # groups (= supertiles)
        self.NCH = 2 * wpc                        # chunks per core


# ----------------------------------------------------------------- host pack

class PackOverflow(Exception):
    pass


def _assign_nodes(pairs, cfg):
    """Balanced node->window map under hard caps (W nodes, CAP pairs)."""
    n_nodes = pairs.shape[0]
    order = np.argsort(-pairs, kind="stable")
    nwin = cfg.NWIN
    heap = [(0, w) for w in range(nwin)]
    counts = np.zeros(nwin, np.int64)     # nodes per window
    loads = np.zeros(nwin, np.int64)      # pair slots per window
    win_of_node = np.full(n_nodes, -1, np.int64)
    for n in order:
        p = int(pairs[n])
        while True:
            if not heap:
                raise PackOverflow("all windows node-full")
            load, w = heapq.heappop(heap)
            if counts[w] < W:
                break
        if load + p > CAP:
            raise PackOverflow(f"window pair overflow: {load}+{p} > {CAP}")
        win_of_node[n] = w
        counts[w] += 1
        loads[w] += p
        if counts[w] < W:
            heapq.heappush(heap, (loads[w], w))
    return win_of_node


def _ef_quantize(row, ea, n_nodes, qdt):
    """Per-(node,feature) error-feedback quantization of edge_attr.

    Returns q (same order as ea, dtype qdt as float32 values) such that
    edges of a node, summed in any grouping, total ~= the fp32 sum."""
    order = np.argsort(row, kind="stable")
    r_s = row[order]
    counts = np.bincount(r_s, minlength=n_nodes)
    starts = np.concatenate([[0], np.cumsum(counts)[:-1]])
    rank = np.arange(len(r_s)) - starts[r_s]
    q_s = np.zeros((len(r_s), ea.shape[1]), np.float32)
    carry = np.zeros((n_nodes, ea.shape[1]), np.float32)
    ea_s = ea[order].astype(np.float32)
    maxdeg = int(rank.max()) + 1 if len(r_s) else 0
    for r in range(maxdeg):
        sel = rank == r
        nodes = r_s[sel]
        v = ea_s[sel] + carry[nodes]
        qv = v.astype(qdt).astype(np.float32)
        q_s[sel] = qv
        carry[nodes] = v - qv
    q = np.empty_like(q_s)
    q[order] = q_s
    return q, order, rank, starts, counts


def _pack(x, edge_index, edge_attr, W1, b1, W2, b2, cfg):
    n_nodes = x.shape[0]
    row = np.asarray(edge_index[0], np.int64)
    deg = np.bincount(row, minlength=n_nodes)
    pairs = (deg + 1) // 2

    win_of_node = _assign_nodes(pairs, cfg)

    # node -> slot inside its window (order of assignment = desc pairs)
    order = np.argsort(-pairs, kind="stable")
    slot_of_node = np.full(n_nodes, -1, np.int64)
    wcount = np.zeros(cfg.NWIN, np.int64)
    # pair-slot offset of each node inside its window
    poff_of_node = np.zeros(n_nodes, np.int64)
    wpload = np.zeros(cfg.NWIN, np.int64)
    for n in order:
        wn = win_of_node[n]
        slot_of_node[n] = wn * W + wcount[wn]
        wcount[wn] += 1
        poff_of_node[n] = wpload[wn]
        wpload[wn] += pairs[n]

    perm = np.full(cfg.NWIN * W, -1, np.int64)
    mask_nodes = slot_of_node >= 0
    perm[slot_of_node[mask_nodes]] = np.arange(n_nodes)[mask_nodes]

    # ---- node features, transposed + permuted, split per core (bf16)
    slots = np.zeros((cfg.NWIN * W, D), np.float32)
    pm = perm >= 0
    slots[pm] = np.asarray(x, np.float32)[perm[pm]]
    xT = np.ascontiguousarray(
        slots.reshape(NCORES, cfg.NPC, D).transpose(0, 2, 1)).astype(BF16_NP)

    # ---- error-feedback fp8 edge quantization
    q, eorder, rank, starts, counts = _ef_quantize(
        row, np.asarray(edge_attr), n_nodes, FP8_NP)

    # ---- route edges into pair slots
    # edge e (rank r within node n): pair index r//2, half r%2
    nodes_s = row[eorder]
    pslot = poff_of_node[nodes_s] + rank // 2          # window pair slot
    half = rank % 2
    wi = win_of_node[nodes_s]
    chunk = pslot // CHUNK                              # 0..CHUNKS-1
    kpos = pslot % CHUNK
    core = wi // cfg.WPC
    wrel = wi % cfg.WPC
    grp = wrel // G
    wg = wrel % G
    cc = wg * CHUNKS + chunk                            # chunk col in group

    edges = np.zeros((NCORES, cfg.NGRP, CHUNK, G * CHUNKS, 2 * D), FP8_NP)
    qv = q[eorder].astype(FP8_NP)
    flat = edges.reshape(-1, 2 * D)
    fidx = ((core * cfg.NGRP + grp) * CHUNK + kpos) * (G * CHUNKS) + cc
    for hh in (0, 1):
        s = half == hh
        flat[fidx[s], hh * D:(hh + 1) * D] = qv[s]

    # ---- relative node ids per pair slot  [core, 128, WPC*CHUNKS]
    rels = np.full((NCORES, CHUNK, cfg.WPC * CHUNKS), 255, BF16_NP)
    # every occupied pair slot: rel id of its node
    rel_id = slot_of_node[nodes_s] % W
    rels[core, kpos, wrel * CHUNKS + chunk] = rel_id.astype(BF16_NP)

    iota = np.ascontiguousarray(
        np.tile(np.arange(W, dtype=np.float32).astype(BF16_NP), (CHUNK, 1)))

    # catT on device holds agg rows on partitions 0:64 and x on 64:128, so
    # swap W1's row halves to match: rows 0:64 must weight agg features.
    W1f = np.asarray(W1, np.float32)
    W1p = np.concatenate([W1f[D:2 * D], W1f[0:D]], axis=0).astype(BF16_NP)
    W2p = np.ascontiguousarray(
        np.asarray(W2, np.float32).reshape(2, 128, O).transpose(1, 0, 2)
        .reshape(128, 2 * O)).astype(BF16_NP)
    b1T = np.ascontiguousarray(
        np.asarray(b1, np.float32).reshape(2, 128).T)      # [128, 2]
    b2c = np.asarray(b2, np.float32).reshape(O, 1)         # [64, 1]

    in_maps = []
    for c in range(NCORES):
        in_maps.append({
            "xT": np.ascontiguousarray(xT[c]),
            "edges": np.ascontiguousarray(edges[c]),
            "rels": np.ascontiguousarray(rels[c]),
            "iota": iota,
            "W1": np.ascontiguousarray(W1p), "W2p": W2p,
            "b1T": b1T, "b2": b2c,
        })
    return in_maps, perm, pm


# -------------------------------------------------------------- device build

def build_nc(cfg, reps=1, skip=frozenset()):
    nc = bacc.Bacc("TRN2", target_bir_lowering=False, debug=False)
    ap_xT = nc.dram_tensor("xT", [D, cfg.NPC], BF16,
                           kind="ExternalInput").ap()
    ap_edges = nc.dram_tensor(
        "edges", [cfg.NGRP, CHUNK, G * CHUNKS, 2 * D], FP8,
        kind="ExternalInput").ap()
    ap_rels = nc.dram_tensor(
        "rels", [CHUNK, cfg.WPC * CHUNKS], BF16, kind="ExternalInput").ap()
    ap_iota = nc.dram_tensor("iota", [CHUNK, W], BF16,
                             kind="ExternalInput").ap()
    ap_W1 = nc.dram_tensor("W1", [2 * D, H], BF16, kind="ExternalInput").ap()
    ap_W2p = nc.dram_tensor("W2p", [H // 2, 2 * O], BF16,
                            kind="ExternalInput").ap()
    ap_b1T = nc.dram_tensor("b1T", [H // 2, 2], F32,
                            kind="ExternalInput").ap()
    ap_b2 = nc.dram_tensor("b2", [O, 1], F32, kind="ExternalInput").ap()
    ap_out = nc.dram_tensor("outT", [O, cfg.NPC], BF16,
                            kind="ExternalOutput").ap()

    AF = mybir.ActivationFunctionType
    with tile.TileContext(nc) as tc, ExitStack() as ctx:
        consts = ctx.enter_context(tc.tile_pool(name="consts", bufs=1))
        epool = ctx.enter_context(tc.tile_pool(name="edges", bufs=4))
        opool = ctx.enter_context(tc.tile_pool(name="onehot", bufs=3))
        hpool = ctx.enter_context(tc.tile_pool(name="hid", bufs=3))
        ypool = ctx.enter_context(tc.tile_pool(name="yout", bufs=2))
        ps_a = ctx.enter_context(tc.tile_pool(name="ps_agg", bufs=3,
                                              space="PSUM"))
        ps_h = ctx.enter_context(tc.tile_pool(name="ps_h", bufs=3,
                                              space="PSUM"))
        ps_o = ctx.enter_context(tc.tile_pool(name="ps_o", bufs=2,
                                              space="PSUM"))

        # catT: partitions 0:64 = aggT (written per supertile), 64:128 = xT
        catT = consts.tile([2 * D, cfg.NPC], BF16)
        nc.sync.dma_start(catT[D:2 * D, :], ap_xT)
        rels = consts.tile([CHUNK, cfg.WPC * CHUNKS], BF16)
        nc.sync.dma_start(rels[:], ap_rels)
        iota = consts.tile([CHUNK, W], BF16)
        nc.sync.dma_start(iota[:], ap_iota)
        W1t = consts.tile([2 * D, H], BF16)
        nc.sync.dma_start(W1t[:], ap_W1)
        W2t = consts.tile([H // 2, 2 * O], BF16)
        nc.sync.dma_start(W2t[:], ap_W2p)
        b1T = consts.tile([H // 2, 2], F32)
        nc.sync.dma_start(b1T[:], ap_b1T)
        b2t = consts.tile([O, 1], F32)
        nc.sync.dma_start(b2t[:], ap_b2)

        NCC = G * CHUNKS   # chunk columns per group

        def mlp(st):
            cat_sl = catT[:, st * ST:(st + 1) * ST]
            hs = []
            for half in range(2):
                w1h = W1t[:, half * 128:(half + 1) * 128]
                h_ps = ps_h.tile([128, ST], F32, tag="h_ps")
                nc.tensor.matmul(h_ps[:], w1h, cat_sl, start=True, stop=True)
                h_sb = hpool.tile([128, ST], BF16, tag="h_sb")
                nc.scalar.activation(h_sb[:], h_ps[:], AF.Relu,
                                     bias=b1T[:, half:half + 1])
                hs.append(h_sb)
            o_ps = ps_o.tile([O, ST], F32)
            nc.tensor.matmul(o_ps[:], W2t[:, 0:O], hs[0][:],
                             start=True, stop=False)
            nc.tensor.matmul(o_ps[:], W2t[:, O:2 * O], hs[1][:],
                             start=False, stop=True)
            o_sb = ypool.tile([O, ST], BF16)
            nc.scalar.activation(o_sb[:], o_ps[:], AF.Identity, bias=b2t[:])
            nc.sync.dma_start(ap_out[:, st * ST:(st + 1) * ST], o_sb[:])

        oh_const = None
        if "tt" in skip:
            oh_const = consts.tile([CHUNK, NCC, W], FP8)
            nc.vector.memset(oh_const[:], 0.0)
        for rep in range(reps):
            for g in range(cfg.NGRP):
                et = epool.tile([CHUNK, NCC, 2 * D], FP8)
                if "edma" not in skip:
                    nc.sync.dma_start(et[:], ap_edges[g])
                elif rep == 0 and g == 0:
                    nc.vector.memset(et[:], 0.0)
                if "tt" not in skip:
                    oh = opool.tile([CHUNK, NCC, W], FP8)
                    rel_bc = (rels[:, g * NCC:(g + 1) * NCC]
                              .unsqueeze(2).broadcast_to([CHUNK, NCC, W]))
                    iota_bc = (iota.unsqueeze(1)
                               .broadcast_to([CHUNK, NCC, W]))
                    nc.vector.tensor_tensor(oh[:], iota_bc, rel_bc,
                                            op=mybir.AluOpType.is_equal)
                else:
                    oh = oh_const
                if "mm" not in skip:
                    a_ps = ps_a.tile([2 * D, ST], F32)
                    for wg in range(G):
                        for c in range(CHUNKS):
                            cc = wg * CHUNKS + c
                            nc.tensor.matmul(a_ps[:, wg * W:(wg + 1) * W],
                                             et[:, cc, :], oh[:, cc, :],
                                             start=(c == 0),
                                             stop=(c == CHUNKS - 1))
                if "fold" not in skip:
                    # psum rows 0:64 + rows 64:128 -> catT agg rows (bf16).
                    # TensorTensor may read only one PSUM operand, so copy
                    # the P half via ACT, then DVE-add the Q half into it.
                    cat_sl = catT[0:D, g * ST:(g + 1) * ST]
                    nc.scalar.activation(cat_sl, a_ps[0:D, :], AF.Copy)
                    nc.vector.tensor_tensor(cat_sl, cat_sl, a_ps[D:2 * D, :],
                                            op=mybir.AluOpType.add)
                if "mlp" not in skip:
                    mlp(g)
    nc.compile()
    return nc


# ------------------------------------------------------------------- driver

_CACHE = {}


def prepare(inputs, reps=1, skip=frozenset()):
    x = np.asarray(inputs["x"])
    edge_index = np.asarray(inputs["edge_index"])
    edge_attr = np.asarray(inputs["edge_attr"])
    row = np.asarray(edge_index[0], np.int64)
    deg = np.bincount(row, minlength=x.shape[0])
    total_pairs = int(((deg + 1) // 2).sum())
    for extra in (0, 16, 32, 64):
        cfg = Cfg(x.shape[0], total_pairs, extra=extra)
        try:
            in_maps, perm, mask = _pack(
                x, edge_index, edge_attr,
                inputs["W1"], inputs["b1"], inputs["W2"], inputs["b2"], cfg)
            break
        except PackOverflow:
            continue
    else:
        raise RuntimeError("could not pack edges into windows")
    key = (cfg.WPC, reps, tuple(sorted(skip)))
    if key not in _CACHE:
        _CACHE[key] = build_nc(cfg, reps=reps, skip=skip)
    return _CACHE[key], in_maps, cfg, perm, mask


def unpack_out(results, cfg, perm, mask, n_nodes):
    slots = np.concatenate(
        [np.asarray(r["outT"]).astype(np.float32).T for r in results], axis=0)
    y = np.zeros((n_nodes, O), np.float32)
    y[perm[perm >= 0]] = slots[perm >= 0]
    return y


def kernel(**inputs):
    nc, in_maps, cfg, perm, mask = prepare(inputs)
    res = run_bass_kernel_spmd(nc, in_maps, list(range(NCORES)))
    return unpack_out(res.results, cfg, perm, mask,
                      np.asarray(inputs["x"]).shape[0])
